# revision 1
# baseline (speedup 1.0000x reference)
import numpy as np

# nn_MixedQLinear: M,K,N = 512,8192,8192; FP_NUM=256; 4-bit asymmetric
# activation quant + int4 weight GEMM + fp outlier path.
# Column-parallel over out_features N across the 8 NeuronCores:
# w_int/fp_weight/weights_scales/reduced_w/bias sharded on N, x replicated.

SHIFT = 8
QMAX = 15


def kernel(x, w_int, fp_weight, bias, weights_scales, reduced_w,
           int_indices, fp_indices):
    import jax
    import jax.numpy as jnp

    devs = jax.devices()
    n_dev = 8 if len(devs) >= 8 else len(devs)
    devs = devs[:n_dev]

    from jax.sharding import Mesh, NamedSharding, PartitionSpec as P
    mesh = Mesh(np.array(devs), ("n",))
    col = NamedSharding(mesh, P("n"))        # shard dim 0 (N rows of weights)
    colT = NamedSharding(mesh, P(None, "n"))  # shard dim 1
    rep = NamedSharding(mesh, P())

    xj = jax.device_put(jnp.asarray(x), rep)
    wij = jax.device_put(jnp.asarray(w_int), col)
    fwj = jax.device_put(jnp.asarray(fp_weight), col)
    bj = jax.device_put(jnp.asarray(bias), col)
    wsj = jax.device_put(jnp.asarray(weights_scales), col)
    rwj = jax.device_put(jnp.asarray(reduced_w), colT)
    iij = jax.device_put(jnp.asarray(int_indices), rep)
    fij = jax.device_put(jnp.asarray(fp_indices), rep)

    @jax.jit
    def f(x, w_int, fp_weight, bias, weights_scales, reduced_w,
          int_indices, fp_indices):
        int_x = jnp.take(x, int_indices, axis=1)
        fp_x = jnp.take(x, fp_indices, axis=1)
        row_min = jnp.min(int_x, axis=1, keepdims=True)
        row_max = jnp.max(int_x, axis=1, keepdims=True)
        scale_row = (row_max - row_min) / QMAX
        q_u = jnp.clip(jnp.round((int_x - row_min) / scale_row), 0, QMAX)
        q_s = q_u - SHIFT
        int_res = jnp.einsum("mk,nk->mn", q_s, w_int.astype(jnp.float32))
        fp_res = jnp.einsum("mf,nf->mn", fp_x, fp_weight) + bias[None, :]
        out = (int_res * scale_row * weights_scales.T
               + (row_min + SHIFT * scale_row) * reduced_w
               + fp_res)
        return out[None]

    out = f(xj, wij, fwj, bj, wsj, rwj, iij, fij)
    return np.asarray(jax.device_get(out)).astype(np.float32)



# revision 7
# speedup vs baseline: 1.0178x; 1.0178x over previous
import numpy as np

# nn_MixedQLinear: M,K,N = 512,8192,8192; FP_NUM=256.
# out = int4quant(x_int) @ w_int^T * scale_row * w_scales
#       + (row_min + 8*scale_row) * reduced_w + x_fp @ fp_w^T + bias
#
# Bass/Tile kernel, column-parallel over N across 8 NeuronCores.
# Key algebra: with unsigned q_u = q_s + 8 in [0,15], the -8 shift terms
# cancel exactly against 8*scale_row*reduced_w (reduced_w = w_scales *
# rowsum(w_int)), giving
#   out = (q_u @ w^T)*scale_row*wscale + row_min*reduced_w + fp_res + bias
# q_u and w are small ints => exact in fp8e4 => fp8 DoubleRow matmul is
# exact.  bias is folded into the fp-outlier GEMM via an appended
# ones-row; the fp operands are pre-divided by wscale (host) and
# scale_row (on chip) so the fp GEMM accumulates into the same PSUM as
# the int GEMM.
# Row stats (min/max) are split across the 8 cores (64 rows each) and
# all-gathered as a tiny [2,64] collective.

M = 512
INT = 7936
NSH = 1024          # out-features per core
NCORES = 8
KT = 62             # 128-wide k tiles
TP = 31             # DoubleRow k pairs (256 wide)
FPK_PAD = 384       # fp-outlier k (256) + ones row (1), zero-padded to 3 tiles
MAGIC = float(2 ** 23)
CHUNKS = [(0, 16), (16, 16), (32, 16), (48, 14)]   # (kt0, nkt)

_CACHE = {}
LAST_EXEC_TIME_NS = None
LAST_MEAN_EXEC_TIME_NS = None


def _ensure_path():
    try:
        import concourse  # noqa: F401
    except ImportError:
        import sys
        for p in ("/opt/trn_rl_repo", "/root/.axon_site/_ro/trn_rl_repo"):
            sys.path.insert(0, p)


def _build_program():
    _ensure_path()
    from contextlib import ExitStack
    import concourse.bass as bass  # noqa: F401
    import concourse.tile as tile
    from concourse import bacc, mybir

    f32 = mybir.dt.float32
    bf16 = mybir.dt.bfloat16
    f8 = mybir.dt.float8e4
    AT = mybir.ActivationFunctionType
    AL = mybir.AluOpType
    AX = mybir.AxisListType
    DR = mybir.MatmulPerfMode.DoubleRow

    nc = bacc.Bacc("TRN2", target_bir_lowering=False, debug=False,
                   num_devices=NCORES)

    x_int_d = nc.dram_tensor("x_int", [M, INT], f32, kind="ExternalInput")
    x_stat_d = nc.dram_tensor("x_stat", [64, INT], f32, kind="ExternalInput")
    xfp_d = nc.dram_tensor("xfp", [FPK_PAD, M], bf16, kind="ExternalInput")
    wdr_d = nc.dram_tensor("wdr", [TP, 128, 2, NSH], f8, kind="ExternalInput")
    fpw_d = nc.dram_tensor("fpw", [FPK_PAD, NSH], bf16, kind="ExternalInput")
    wsc_d = nc.dram_tensor("wsc", [1, NSH], f32, kind="ExternalInput")
    rw_d = nc.dram_tensor("rw", [1, NSH], f32, kind="ExternalInput")
    ident_d = nc.dram_tensor("ident", [128, 128], f32, kind="ExternalInput")
    out_d = nc.dram_tensor("out", [M, NSH], f32, kind="ExternalOutput")

    with tile.TileContext(nc) as tc, ExitStack() as ctx:
        cpool = ctx.enter_context(tc.tile_pool(name="consts", bufs=1))
        dpool = ctx.enter_context(tc.tile_pool(name="dram", bufs=1, space="DRAM"))
        statsp = ctx.enter_context(tc.tile_pool(name="stats", bufs=2))
        xpool = ctx.enter_context(tc.tile_pool(name="x", bufs=2))
        qfpool = ctx.enter_context(tc.tile_pool(name="qf", bufs=2))
        q8pool = ctx.enter_context(tc.tile_pool(name="q8", bufs=3))
        qtpool = ctx.enter_context(tc.tile_pool(name="qt", bufs=1))
        wpool = ctx.enter_context(tc.tile_pool(name="w", bufs=TP))
        pspool = ctx.enter_context(tc.tile_pool(name="ps", bufs=8, space="PSUM"))
        opool = ctx.enter_context(tc.tile_pool(name="o", bufs=3))

        sync, gps, v, sc, pe = nc.sync, nc.gpsimd, nc.vector, nc.scalar, nc.tensor

        # ---- constants ----
        identf = cpool.tile([128, 128], f32, tag="identf")
        sync.dma_start(identf[:], ident_d[:, :])
        ident8 = cpool.tile([128, 128], f8, tag="ident8")
        v.tensor_copy(ident8[:], identf[:])

        wscs = cpool.tile([1, NSH], f32, tag="wscs")
        sync.dma_start(wscs[:], wsc_d[:, :])
        rws = cpool.tile([1, NSH], f32, tag="rws")
        sync.dma_start(rws[:], rw_d[:, :])
        wsc_b = cpool.tile([128, NSH], f32, tag="wsc_b")
        gps.partition_broadcast(wsc_b[:], wscs[:])
        rw_b = cpool.tile([128, NSH], f32, tag="rw_b")
        gps.partition_broadcast(rw_b[:], rws[:])

        xfp_raw, fpw_sb = [], []
        for kf in range(3):
            t = cpool.tile([128, M], bf16, tag=f"xfpr{kf}")
            sync.dma_start(t[:], xfp_d[128 * kf:128 * (kf + 1), :])
            xfp_raw.append(t)
            t2 = cpool.tile([128, NSH], bf16, tag=f"fpw{kf}")
            sync.dma_start(t2[:], fpw_d[128 * kf:128 * (kf + 1), :])
            fpw_sb.append(t2)

        # ---- local row stats over this core's 64 rows ----
        SH = INT // 2  # 3968: rows doubled onto 128 partitions
        st_u = []
        for u in range(2):
            stc = statsp.tile([128, SH // 2], f32, tag="statx")
            for h in range(2):
                sync.dma_start(
                    stc[64 * h:64 * (h + 1), :],
                    x_stat_d[0:64, SH * h + (SH // 2) * u:
                             SH * h + (SH // 2) * (u + 1)])
            st = cpool.tile([128, 2], f32, tag=f"st{u}")
            v.tensor_reduce(st[:, 0:1], stc[:], AX.X, AL.min)
            # store -max so every later combine is a min (partition-0 APs)
            v.tensor_reduce(st[:, 1:2], stc[:], AX.X, AL.max, negate=True)
            st_u.append(st)
        stf = cpool.tile([128, 2], f32, tag="stf")
        v.tensor_tensor(stf[:], st_u[0][:], st_u[1][:], AL.min)

        ps_st = pspool.tile([2, 128], f32, tag="ps")
        pe.matmul(ps_st[:], lhsT=stf[:], rhs=identf[:], start=True, stop=True)
        stl = cpool.tile([2, 128], f32, tag="stl")
        v.tensor_copy(stl[:], ps_st[:])
        sb_loc = cpool.tile([2, 64], f32, tag="sb_loc")
        v.tensor_tensor(sb_loc[:, :], stl[:, 0:64], stl[:, 64:128], AL.min)

        stats_loc = dpool.tile([2, 64], f32, tag="stats_loc")
        stats_g = dpool.tile([2 * NCORES, 64], f32, tag="stats_g")
        gps.dma_start(stats_loc[:], sb_loc[:])
        gps.collective_compute(
            "AllGather", AL.bypass,
            replica_groups=[list(range(NCORES))],
            ins=[stats_loc.opt()], outs=[stats_g.opt()])

        # gather back: free layout [1,512] and per-partition [128, 8]
        sgap = stats_g[:, :]
        r1 = sgap.rearrange("(c t) j -> t c j", t=2)
        minf = cpool.tile([1, M], f32, tag="minf")
        nmaxf = cpool.tile([1, M], f32, tag="nmaxf")
        gps.dma_start(minf[:], r1[0:1])
        gps.dma_start(nmaxf[:], r1[1:2])
        stats_pp = cpool.tile([128, 2, 4], f32, tag="stats_pp")
        r2 = sgap.rearrange("(mt half t) j -> half j t mt", mt=4, half=2, t=2)
        for h in range(2):
            for t in range(2):
                gps.dma_start(stats_pp[64 * h:64 * (h + 1), t, :],
                              r2[h][:, t, :])

        # derived per-partition stats [128, 4] (col = m-tile)
        min_pp = stats_pp[:, 0, :]
        nmax_pp = stats_pp[:, 1, :]
        rng_pp = cpool.tile([128, 4], f32, tag="rng_pp")
        v.scalar_tensor_tensor(rng_pp[:], nmax_pp, -1.0, min_pp,
                               AL.mult, AL.subtract)
        rec_pp = cpool.tile([128, 4], f32, tag="rec_pp")
        v.reciprocal(rec_pp[:], rng_pp[:])
        inv15_pp = cpool.tile([128, 4], f32, tag="inv15_pp")
        v.tensor_scalar(inv15_pp[:], rec_pp[:], 15.0, None, AL.mult)
        s_pp = cpool.tile([128, 4], f32, tag="s_pp")
        v.tensor_scalar(s_pp[:], rng_pp[:], 1.0 / 15.0, None, AL.mult)
        nmo_pp = cpool.tile([128, 4], f32, tag="nmo_pp")
        v.scalar_tensor_tensor(nmo_pp[:], min_pp, -1.0, inv15_pp[:],
                               AL.mult, AL.mult)

        # free-layout 15/rng, broadcast for the fp-path scaling
        rngf = cpool.tile([1, M], f32, tag="rngf")
        v.scalar_tensor_tensor(rngf[:], nmaxf[:], -1.0, minf[:],
                               AL.mult, AL.subtract)
        recf = cpool.tile([1, M], f32, tag="recf")
        v.reciprocal(recf[:], rngf[:])
        invf = cpool.tile([1, M], f32, tag="invf")
        v.tensor_scalar(invf[:], recf[:], 15.0, None, AL.mult)
        invfb = cpool.tile([1, M], bf16, tag="invfb")
        v.tensor_copy(invfb[:], invf[:])
        inv_b = cpool.tile([128, M], bf16, tag="inv_b")
        gps.partition_broadcast(inv_b[:], invfb[:])

        xfp_s = []
        for kf in range(3):
            t = cpool.tile([128, M], bf16, tag=f"xfps{kf}")
            v.tensor_tensor(t[:], xfp_raw[kf][:], inv_b[:], AL.mult)
            xfp_s.append(t)

        # ---- weights (kept resident in SBUF) ----
        wts = []
        for tp in range(TP):
            wts.append(wpool.tile([128, 2, NSH], f8, tag="wt", name=f"wt{tp}"))

        qT = qtpool.tile([128, KT, 4, 128], f8, tag="qT")

        # PSUM accumulators for m-tiles 0..2 (phase 1); m-tile 3 in phase 2
        acc = {}
        for mt in range(3):
            for nh in range(2):
                acc[(mt, nh)] = pspool.tile([128, 512], f32, tag="ps", name=f"acc{mt}_{nh}")

        def epilogue(mt, nh, acct):
            e1 = opool.tile([128, 512], f32, tag="e1")
            v.scalar_tensor_tensor(e1[:], acct[:], s_pp[:, mt:mt + 1],
                                   wsc_b[:, 512 * nh:512 * (nh + 1)],
                                   AL.mult, AL.mult)
            ot = opool.tile([128, 512], f32, tag="ot")
            v.scalar_tensor_tensor(ot[:], rw_b[:, 512 * nh:512 * (nh + 1)],
                                   stats_pp[:, 0, mt:mt + 1], e1[:],
                                   AL.mult, AL.add)
            sc.dma_start(out_d[128 * mt:128 * (mt + 1),
                               512 * nh:512 * (nh + 1)], ot[:])

        cp_i = 0
        for kci, (kt0, nkt) in enumerate(CHUNKS):
            # weights for this chunk
            for tp in range(kt0 // 2, (kt0 + nkt) // 2):
                sync.dma_start(wts[tp][:], wdr_d[tp])
            c0, csz = kt0 * 128, nkt * 128
            for mt in range(4):
                xt = xpool.tile([128, 2048], f32, tag="xt")
                sync.dma_start(xt[:, 0:csz],
                               x_int_d[128 * mt:128 * (mt + 1), c0:c0 + csz])
                qf = qfpool.tile([128, 2048], f32, tag="qf")
                sc.activation(qf[:, 0:csz], xt[:, 0:csz], AT.Identity,
                              bias=nmo_pp[:, mt:mt + 1],
                              scale=inv15_pp[:, mt:mt + 1])
                q8 = q8pool.tile([128, 2048], f8, tag="q8")
                v.tensor_scalar(q8[:, 0:csz], qf[:, 0:csz], MAGIC, -MAGIC,
                                AL.add, AL.add)
                # transpose this m-tile's chunk into qT
                for b0 in range(0, nkt, 4):
                    bn = min(4, nkt - b0)
                    pst = pspool.tile([128, 512], f32, tag="ps")
                    for u in range(bn):
                        pe.matmul(pst[:, 128 * u:128 * (u + 1)],
                                  lhsT=q8[:, 128 * (b0 + u):128 * (b0 + u + 1)],
                                  rhs=ident8[:], start=True, stop=True)
                    dst = qT[:, kt0 + b0:kt0 + b0 + bn, mt, :]
                    src = pst[:, 0:128 * bn]
                    if cp_i % 2 == 0:
                        sc.copy(dst, src)
                    else:
                        v.tensor_copy(dst, src)
                    cp_i += 1
            # int GEMM for m-tiles 0..2 over this chunk's k pairs
            for tp in range(kt0 // 2, (kt0 + nkt) // 2):
                for mt in range(3):
                    for nh in range(2):
                        pe.matmul(acc[(mt, nh)][:],
                                  lhsT=qT[:, 2 * tp:2 * tp + 2, mt, :],
                                  rhs=wts[tp][:, :, 512 * nh:512 * (nh + 1)],
                                  start=(tp == 0), stop=False,
                                  perf_mode=DR, skip_group_check=True)

        # fp outlier GEMM + epilogue, m-tiles 0..2
        for mt in range(3):
            for nh in range(2):
                for kf in range(3):
                    pe.matmul(acc[(mt, nh)][:],
                              lhsT=xfp_s[kf][:, 128 * mt:128 * (mt + 1)],
                              rhs=fpw_sb[kf][:, 512 * nh:512 * (nh + 1)],
                              start=False, stop=(kf == 2),
                              skip_group_check=True)
                epilogue(mt, nh, acc[(mt, nh)])

        # m-tile 3 (PSUM slots freed by transpose staging)
        acc3 = {nh: pspool.tile([128, 512], f32, tag="ps", name=f"acc3_{nh}")
            for nh in range(2)}
        for tp in range(TP):
            for nh in range(2):
                pe.matmul(acc3[nh][:],
                          lhsT=qT[:, 2 * tp:2 * tp + 2, 3, :],
                          rhs=wts[tp][:, :, 512 * nh:512 * (nh + 1)],
                          start=(tp == 0), stop=False,
                          perf_mode=DR, skip_group_check=True)
        for nh in range(2):
            for kf in range(3):
                pe.matmul(acc3[nh][:],
                          lhsT=xfp_s[kf][:, 384:512],
                          rhs=fpw_sb[kf][:, 512 * nh:512 * (nh + 1)],
                          start=False, stop=(kf == 2), skip_group_check=True)
            epilogue(3, nh, acc3[nh])

    nc.compile()
    return nc


def _host_prep(x, w_int, fp_weight, bias, weights_scales, reduced_w,
               int_indices, fp_indices):
    import ml_dtypes
    bf16 = ml_dtypes.bfloat16
    f8 = ml_dtypes.float8_e4m3

    x = np.asarray(x, np.float32)
    ii = np.asarray(int_indices, np.int64)
    fi = np.asarray(fp_indices, np.int64)
    w_int = np.asarray(w_int)
    fp_weight = np.asarray(fp_weight, np.float32)
    bias = np.asarray(bias, np.float32)
    ws = np.asarray(weights_scales, np.float32).reshape(-1)     # [N]
    rw = np.asarray(reduced_w, np.float32).reshape(-1)          # [N]

    x_int = np.ascontiguousarray(x[:, ii])                      # [512, 7936]
    x_fp = x[:, fi]                                             # [512, 256]

    xfp_ext = np.zeros((FPK_PAD, M), dtype=bf16)
    xfp_ext[0:256] = x_fp.T.astype(bf16)
    xfp_ext[256] = bf16(1.0)

    ident = np.eye(128, dtype=np.float32)

    in_maps = []
    for c in range(NCORES):
        sl = slice(c * NSH, (c + 1) * NSH)
        wsh = w_int[sl].astype(np.float32)                      # [1024, 7936]
        wT = np.ascontiguousarray(wsh.T).astype(f8)             # [7936, 1024]
        wdr = np.ascontiguousarray(
            wT.reshape(TP, 2, 128, NSH).transpose(0, 2, 1, 3))  # [31,128,2,1024]
        wsc = ws[sl]
        fpw_ext = np.zeros((FPK_PAD, NSH), dtype=bf16)
        fpw_ext[0:256] = (fp_weight[sl] / wsc[:, None]).T.astype(bf16)
        fpw_ext[256] = (bias[sl] / wsc).astype(bf16)
        in_maps.append({
            "x_int": x_int,
            "x_stat": np.ascontiguousarray(x_int[64 * c:64 * (c + 1)]),
            "xfp": xfp_ext,
            "wdr": wdr,
            "fpw": fpw_ext,
            "wsc": np.ascontiguousarray(wsc.reshape(1, NSH)),
            "rw": np.ascontiguousarray(rw[sl].reshape(1, NSH)),
            "ident": ident,
        })
    return in_maps


def kernel(x, w_int, fp_weight, bias, weights_scales, reduced_w,
           int_indices, fp_indices):
    global LAST_EXEC_TIME_NS, LAST_MEAN_EXEC_TIME_NS
    _ensure_path()
    from concourse.bass_utils import run_bass_kernel_spmd

    if "nc" not in _CACHE:
        _CACHE["nc"] = _build_program()
    nc = _CACHE["nc"]

    in_maps = _host_prep(x, w_int, fp_weight, bias, weights_scales,
                         reduced_w, int_indices, fp_indices)
    res = run_bass_kernel_spmd(nc, in_maps, list(range(NCORES)))
    LAST_EXEC_TIME_NS = res.exec_time_ns
    LAST_MEAN_EXEC_TIME_NS = res.mean_exec_time_ns

    out = np.concatenate([res.results[c]["out"] for c in range(NCORES)],
                         axis=1).astype(np.float32)
    return out[None]
